# revision 2
# baseline (speedup 1.0000x reference)
"""JPEG layer (nn_JpegLayer) Trainium2 Bass kernel, 8-core data parallel.

Pipeline per image (per core: 4 images of [3,512,512]):
  P1: 3-accum matmuls fold RGB->YCC color mix + H-DCT (+ vertical 2x-pool for
      chroma) ; route-A, out [h'freq, w]
  T1: PE transposes -> [w, h'freq]
  P2: W-DCT (+ horizontal pool fold for chroma) + DC level-shift correction
      via an extra accumulated rank-structured matmul -> coeffs [w'', h']
  Q : e = d*(1/q); round via +/-2^23*1.5 trick; dec = r*q   (DVE/GPSIMD)
  P3: W-IDCT (+ horizontal 2x upsample fold for chroma) -> [w, h']
  T2: PE transposes -> [h', w]
  P4: H-IDCT (+ vertical upsample fold for chroma) + YCC->RGB fold via
      accumulated matmuls + LEVEL plane via ones-matmul -> psum RGB
  out: DVE tensor_scalar (max 0, min 1) psum->sbuf, DMA out.

All matmul data is float32r (TRN2 reduced-precision fp32 path, 1 cyc/row at
N>=256). Forward-path rounding error ~1e-4 rel; set FP32_FWD=True to run the
forward passes in full fp32 (4 cyc/row) if more accuracy is needed.
"""
import sys
sys.path.insert(0, '/opt/trn_rl_repo')
import numpy as np
import concourse.bacc as bacc
import concourse.bass as bass
import concourse.mybir as mybir
import concourse.tile as tile
from concourse import bass_utils

N_CORES = 8
IMG_PER_CORE = 4
H = W = 512
HT = H // 128            # 4 h-tiles per plane
LEVEL = np.float32(128.0 / 255.0)
LEVEL_F = float(LEVEL)
C_ROUND = 12582912.0   # 1.5*2^23: (x+C)-C == round-half-even(x)
F32 = mybir.dt.float32
F32R = mybir.dt.float32r

RGB2YCC = np.array([[0.299, 0.587, 0.114],
                    [-0.168735892, -0.331264108, 0.5],
                    [0.5, -0.418687589, -0.081312411]], dtype=np.float32)
# YCC2RGB columns: Y col = [1,1,1]; cb col = [0,-0.344136286,1.772]; cr col = [1.402,-0.714136286,0]
CB_C = np.array([0.0, -0.344136286, 1.772], dtype=np.float32)
CR_C = np.array([1.402, -0.714136286, 0.0], dtype=np.float32)


def _dct8():
    i = np.arange(8)[:, None].astype(np.float64)
    j = np.arange(8)[None, :].astype(np.float64)
    m = np.sqrt(2.0 / 8) * np.cos(np.pi * (2 * j + 1) * i / 16.0)
    m[0, :] = 1.0 / np.sqrt(8.0)
    return m.astype(np.float32)


def _blockdiag(b, reps):
    r, c = b.shape
    out = np.zeros((r * reps, c * reps), dtype=np.float32)
    for k in range(reps):
        out[k * r:(k + 1) * r, k * c:(k + 1) * c] = b
    return out


def _build_consts(quantize):
    D = _dct8()
    BD_T = _blockdiag(D.T, 16)             # [128,128] fwd 1D-DCT as lhsT
    BD = _blockdiag(D, 16)                 # [128,128] inverse
    # pooled fwd: PF[16b+2ii+dh, 8b+u] = D[u,ii]/2    [128, 64]
    pf8 = np.zeros((16, 8), dtype=np.float32)
    for ii in range(8):
        for dh in range(2):
            pf8[2 * ii + dh, :] = D[:, ii] * 0.5
    PF = _blockdiag(pf8, 8)                # [128, 64]
    # upsample inverse: PU[8b+v, 16b+2jj+dw] = D[v,jj]   [64, 128]
    pu8 = np.zeros((8, 16), dtype=np.float32)
    for jj in range(8):
        for dw in range(2):
            pu8[:, 2 * jj + dw] = D[jj, :]     # D.T[v,jj] = D[jj,v]? no:
    # careful: idct y[j] = sum_v D[v,j] z[v]  => PU[v, col(j,dw)] = D[v, j]
    pu8 = np.zeros((8, 16), dtype=np.float32)
    for jj in range(8):
        for dw in range(2):
            pu8[:, 2 * jj + dw] = D[:, jj]
    PU = _blockdiag(pu8, 8)                # [64, 128]

    consts = {}
    for c in range(3):
        consts[f"w1y{c}"] = RGB2YCC[0, c] * BD_T
        consts[f"w1c{c}"] = np.concatenate(
            [RGB2YCC[1, c] * PF, RGB2YCC[2, c] * PF], axis=1)  # [128,128]
    consts["w2y"] = BD_T
    consts["w2c"] = PF                     # [128, 64]
    consts["w3y"] = BD
    consts["w3c"] = PU                     # [64, 128]
    consts["w4y"] = BD
    w4 = {}
    for name, cb, cr in (("R", CB_C[0], CR_C[0]), ("G", CB_C[1], CR_C[1]),
                         ("B", CB_C[2], CR_C[2])):
        m = np.zeros((128, 128), dtype=np.float32)
        m[0:64, :] = cb * PU
        m[64:128, :] = cr * PU
        consts[f"w4c{name}"] = m
    consts["ident"] = np.eye(128, dtype=np.float32)

    # quant tables: q = round(quantize[0]*255)/255 (f32, all channels)
    q = (np.round(quantize[0].astype(np.float32) * np.float32(255.0))
         / np.float32(255.0)).astype(np.float32)
    rq = (1.0 / q.astype(np.float64)).astype(np.float32)
    consts["rqt"] = np.tile(rq.T, (16, 64)).astype(np.float32)   # [128,512]
    consts["qt"] = np.tile(q.T, (16, 64)).astype(np.float32)
    # DC correction: coeff d_true = d - 8L*delta00. Via accumulated matmul:
    # lhsT dccor [128,128]: col p (p%8==0) = -8L/128 ; rhs pat8 [128,512]:
    # pat8[k, n] = 1 if n%8==0 else 0  -> psum[p,n] += -8L*d(p%8=0)*d(n%8=0)
    dccor = np.zeros((128, 128), dtype=np.float32)
    dccor[:, 0::8] = np.float32(-8.0 * LEVEL / 128.0)
    consts["dccor"] = dccor
    pat8 = np.zeros((128, 512), dtype=np.float32)
    pat8[:, 0::8] = 1.0
    consts["pat8"] = pat8
    # LEVEL plane: lhsT lones [128,128] all L/128, rhs ones [128,512]
    consts["lones"] = np.full((128, 128), LEVEL / np.float32(128.0),
                              dtype=np.float32)
    consts["ones"] = np.ones((128, 512), dtype=np.float32)
    return consts


_CONST_SHAPES = None


def _build_nc():
    nc = bacc.Bacc("TRN2", target_bir_lowering=False, debug=False,
                   enable_asserts=False, num_devices=N_CORES)
    x_d = nc.dram_tensor("x", [IMG_PER_CORE, 3, H, W], F32R,
                         kind="ExternalInput").ap()
    out_d = nc.dram_tensor("out", [IMG_PER_CORE, 3, H, W], F32,
                           kind="ExternalOutput").ap()
    cd = {}
    for name, shape in _CONST_SHAPES.items():
        cd[name] = nc.dram_tensor(name, list(shape), F32R,
                                  kind="ExternalInput").ap()

    with tile.TileContext(nc) as tc:
        with tc.tile_pool(name="consts", bufs=1) as cp, \
             tc.tile_pool(name="xin", bufs=14) as xp, \
             tc.tile_pool(name="work", bufs=5) as wp, \
             tc.tile_pool(name="stage", bufs=4) as sp, \
             tc.tile_pool(name="psmm", bufs=2, space="PSUM") as pmm, \
             tc.tile_pool(name="pstp", bufs=2, space="PSUM") as ptp:

            cs = {}
            for name, shape in _CONST_SHAPES.items():
                cs[name] = cp.tile(list(shape), F32R, tag=f"c_{name}", name=f"c_{name}")
                nc.sync.dma_start(cs[name][:], cd[name])

            ACT = mybir.ActivationFunctionType
            OP = mybir.AluOpType

            for img in range(IMG_PER_CORE):
                # ---- load RGB tiles ----
                X = {}
                for c in range(3):
                    for t in range(HT):
                        xt = xp.tile([128, 512], F32R, tag="x", name=f"x_{img}_{c}_{t}")
                        nc.sync.dma_start(
                            xt[:], x_d[img, c, 128 * t:128 * (t + 1), :])
                        X[c, t] = xt

                # ---- P1: color + H-DCT (+v-pool chroma) ----
                d1y, d1c = [], []
                for t in range(HT):
                    psY = pmm.tile([128, 512], F32, tag="mm", name="psmm_t")
                    for c in range(3):
                        nc.tensor.matmul(psY[:], cs[f"w1y{c}"][:], X[c, t][:],
                                         start=(c == 0), stop=(c == 2))
                    ty = wp.tile([128, 512], F32R, tag="d1y", name=f"d1y_{img}_{t}")
                    nc.scalar.activation(ty[:], psY[:], ACT.Copy)
                    d1y.append(ty)
                    psC = pmm.tile([128, 512], F32, tag="mm", name="psmm_t")
                    for c in range(3):
                        nc.tensor.matmul(psC[:], cs[f"w1c{c}"][:], X[c, t][:],
                                         start=(c == 0), stop=(c == 2))
                    tcc = wp.tile([128, 512], F32R, tag="d1c", name=f"d1c_{img}_{t}")
                    nc.vector.tensor_copy(tcc[:], psC[:])
                    d1c.append(tcc)

                # ---- T1 ----
                t1y, t1c = [], []
                for s in range(4):
                    pty = ptp.tile([128, 512], F32R, tag="tp", name="pstp_t")
                    for t in range(HT):
                        nc.tensor.transpose(
                            pty[:, 128 * t:128 * (t + 1)],
                            d1y[t][:, 128 * s:128 * (s + 1)], cs["ident"][:])
                    sy = wp.tile([128, 512], F32R, tag="t1y", name=f"t1y_{img}_{s}")
                    nc.scalar.activation(sy[:], pty[:], ACT.Copy)
                    t1y.append(sy)
                    ptc = ptp.tile([128, 512], F32R, tag="tp", name="pstp_t")
                    for t in range(HT):
                        nc.tensor.transpose(
                            ptc[:, 128 * t:128 * (t + 1)],
                            d1c[t][:, 128 * s:128 * (s + 1)], cs["ident"][:])
                    sc = wp.tile([128, 512], F32R, tag="t1c", name=f"t1c_{img}_{s}")
                    nc.vector.tensor_copy(sc[:], ptc[:])
                    t1c.append(sc)

                # ---- P2 + quantize ----
                decy, decc = [], []
                for s in range(4):
                    ps = pmm.tile([128, 512], F32, tag="mm", name="psmm_t")
                    nc.tensor.matmul(ps[:], cs["w2y"][:], t1y[s][:],
                                     start=True, stop=False)
                    nc.tensor.matmul(ps[:], cs["dccor"][:], cs["pat8"][:],
                                     start=False, stop=True)
                    ey = wp.tile([128, 512], F32R, tag="ey", name=f"ey_{img}_{s}")
                    nc.vector.tensor_tensor(ey[:], ps[:], cs["rqt"][:], OP.mult)
                    nc.gpsimd.tensor_scalar(ey[:], ey[:], C_ROUND, C_ROUND,
                                            OP.add, OP.subtract)
                    dy = wp.tile([128, 512], F32R, tag="decy", name=f"decy_{img}_{s}")
                    nc.vector.tensor_tensor(dy[:], ey[:], cs["qt"][:], OP.mult)
                    decy.append(dy)

                    psc = pmm.tile([64, 512], F32, tag="mmc", name="psmmc_t")
                    nc.tensor.matmul(psc[:], cs["w2c"][:], t1c[s][:],
                                     start=True, stop=True)
                    ec = wp.tile([64, 512], F32R, tag="ec", name=f"ec_{img}_{s}")
                    nc.vector.tensor_tensor(ec[:], psc[:], cs["rqt"][0:64, :],
                                            OP.mult)
                    nc.gpsimd.tensor_scalar(ec[:], ec[:], C_ROUND, C_ROUND,
                                            OP.add, OP.subtract)
                    dc = wp.tile([64, 512], F32R, tag="decc", name=f"decc_{img}_{s}")
                    nc.vector.tensor_tensor(dc[:], ec[:], cs["qt"][0:64, :],
                                            OP.mult)
                    decc.append(dc)

                # ---- P3 ----
                p3y, p3c = [], []
                for s in range(4):
                    ps = pmm.tile([128, 512], F32, tag="mm", name="psmm_t")
                    nc.tensor.matmul(ps[:], cs["w3y"][:], decy[s][:],
                                     start=True, stop=True)
                    vy = wp.tile([128, 512], F32R, tag="p3y", name=f"p3y_{img}_{s}")
                    nc.scalar.activation(vy[:], ps[:], ACT.Copy)
                    p3y.append(vy)
                    psc = pmm.tile([128, 512], F32, tag="mm", name="psmm_t")
                    nc.tensor.matmul(psc[:], cs["w3c"][:], decc[s][:],
                                     start=True, stop=True)
                    vc = wp.tile([128, 512], F32R, tag="p3c", name=f"p3c_{img}_{s}")
                    nc.scalar.activation(vc[:], psc[:], ACT.Copy)
                    p3c.append(vc)

                # ---- T2 ----
                t2y, t2c = [], []
                for t in range(4):
                    pty = ptp.tile([128, 512], F32R, tag="tp", name="pstp_t")
                    for s in range(4):
                        nc.tensor.transpose(
                            pty[:, 128 * s:128 * (s + 1)],
                            p3y[s][:, 128 * t:128 * (t + 1)], cs["ident"][:])
                    sy = wp.tile([128, 512], F32R, tag="t2y", name=f"t2y_{img}_{t}")
                    nc.scalar.activation(sy[:], pty[:], ACT.Copy)
                    t2y.append(sy)
                    ptc = ptp.tile([128, 512], F32R, tag="tp", name="pstp_t")
                    for s in range(4):
                        nc.tensor.transpose(
                            ptc[:, 128 * s:128 * (s + 1)],
                            p3c[s][:, 128 * t:128 * (t + 1)], cs["ident"][:])
                    sc = wp.tile([128, 512], F32R, tag="t2c", name=f"t2c_{img}_{t}")
                    nc.vector.tensor_copy(sc[:], ptc[:])
                    t2c.append(sc)

                # ---- P4 + color back + LEVEL + clamp + store ----
                for t in range(4):
                    for ci, cname in enumerate(("R", "G", "B")):
                        ps = pmm.tile([128, 512], F32, tag="mm", name="psmm_t")
                        nc.tensor.matmul(ps[:], cs["w4y"][:], t2y[t][:],
                                         start=True, stop=False)
                        nc.tensor.matmul(ps[:], cs[f"w4c{cname}"][:], t2c[t][:],
                                         start=False, stop=False)
                        nc.tensor.matmul(ps[:], cs["lones"][:], cs["ones"][:],
                                         start=False, stop=True)
                        og = sp.tile([128, 512], F32, tag="og", name=f"og_{img}_{t}_{ci}")
                        nc.vector.tensor_scalar(og[:], ps[:], 0.0, 1.0,
                                                OP.max, OP.min)
                        nc.sync.dma_start(
                            out_d[img, ci, 128 * t:128 * (t + 1), :], og[:])
    nc.compile()
    return nc


_NC_CACHE = None


def kernel(input, quantize):
    global _NC_CACHE, _CONST_SHAPES
    input = np.asarray(input, dtype=np.float32)
    quantize = np.asarray(quantize, dtype=np.float32)
    consts = _build_consts(quantize)
    if _CONST_SHAPES is None:
        _CONST_SHAPES = {k: v.shape for k, v in consts.items()}
    if _NC_CACHE is None:
        _NC_CACHE = _build_nc()
    nc = _NC_CACHE

    in_maps = []
    for core in range(N_CORES):
        shard = np.ascontiguousarray(
            input[core * IMG_PER_CORE:(core + 1) * IMG_PER_CORE])
        m = {"x": shard}
        m.update(consts)
        in_maps.append(m)
    res = bass_utils.run_bass_kernel_spmd(nc, in_maps,
                                          core_ids=list(range(N_CORES)))
    global LAST_RESULT
    LAST_RESULT = res
    out = np.concatenate([res.results[i]["out"] for i in range(N_CORES)],
                         axis=0)
    return out.astype(np.float32)


LAST_RESULT = None



# revision 21
# speedup vs baseline: 3.6543x; 3.6543x over previous
"""JPEG layer (nn_JpegLayer) Trainium2 Bass kernel, 8-core data parallel, v2.

Per core: 4 images of [3,512,512]. Pipeline (per image):
  S1  : fused color-mix + H-DCT + transpose via data-as-lhsT matmuls
        (out = X_chunk^T @ [w1y_c | w1c_c], N=256, fp32r) -> t1 [w, hfreq]
  P2  : W-DCT (y: + level-shift DC correction via accumulated rank matmul;
        c: s-pair packed into 128 partitions via [PF|0]/[0|PF] lhsT)
  Q   : e = d*(1/q) (DVE) ; round via +/-1.5*2^23 (DVE/Act) ; dec = r*q
        (DVE, out bf16)
  T2P3: fused W-IDCT + transpose via data-as-lhsT matmuls in bf16
        (1 cyc/row at any N) ; +LEVEL folded as per-partition bias on the
        psum->sbuf copy of the y tiles
  P4  : H-IDCT + color mix (2 accumulated matmuls per color, bf16 moving)
  out : clamp [0,1] on DVE, batched 3-channel DMA out.

GPSIMD is not used anywhere (9us/instr measured on HW). Elementwise work is
split across DVE and Act; all matmul stages keep the moving dim >= 256 in
fp32r or use bf16 (1 cyc/row at any N).
"""
import sys
sys.path.insert(0, '/opt/trn_rl_repo')
import numpy as np
import ml_dtypes
import concourse.bacc as bacc
import concourse.bass as bass
import concourse.mybir as mybir
import concourse.tile as tile
from concourse import bass_utils

N_CORES = 8
IMG_PER_CORE = 4
H = W = 512
LEVEL = np.float32(128.0 / 255.0)
C_ROUND = 12582912.0   # 1.5*2^23: (x+C)-C == round-half-even(x)
F32 = mybir.dt.float32
F32R = mybir.dt.float32r
BF16 = mybir.dt.bfloat16

RGB2YCC = np.array([[0.299, 0.587, 0.114],
                    [-0.168735892, -0.331264108, 0.5],
                    [0.5, -0.418687589, -0.081312411]], dtype=np.float64)
CB_C = np.array([0.0, -0.344136286, 1.772])
CR_C = np.array([1.402, -0.714136286, 0.0])


def _dct8():
    i = np.arange(8)[:, None].astype(np.float64)
    j = np.arange(8)[None, :].astype(np.float64)
    m = np.sqrt(2.0 / 8) * np.cos(np.pi * (2 * j + 1) * i / 16.0)
    m[0, :] = 1.0 / np.sqrt(8.0)
    return m


def _blockdiag(b, reps):
    r, c = b.shape
    out = np.zeros((r * reps, c * reps))
    for k in range(reps):
        out[k * r:(k + 1) * r, k * c:(k + 1) * c] = b
    return out


def _f32(a):
    return np.ascontiguousarray(a, dtype=np.float32)


def _bf16(a):
    return np.ascontiguousarray(np.asarray(a, dtype=np.float32)
                                .astype(ml_dtypes.bfloat16))


def _build_consts(quantize):
    D = _dct8()
    BD_T = _blockdiag(D.T, 16)         # [128 pix, 128 freq] forward
    BD = _blockdiag(D, 16)             # [128 freq, 128 pix] inverse
    pf8 = np.zeros((16, 8))
    for ii in range(8):
        for dh in range(2):
            pf8[2 * ii + dh, :] = D[:, ii] * 0.5
    PF = _blockdiag(pf8, 8)            # [128 pix, 64 freq'] fwd pooled
    pu8 = np.zeros((8, 16))
    for jj in range(8):
        for dw in range(2):
            pu8[:, 2 * jj + dw] = D[:, jj]
    PU = _blockdiag(pu8, 8)            # [64 freq', 128 pix] inv upsampled

    consts = {}
    for c in range(3):
        w1c = np.concatenate([RGB2YCC[1, c] * PF, RGB2YCC[2, c] * PF], axis=1)
        consts[f"r{c}"] = (_f32(np.concatenate(
            [RGB2YCC[0, c] * BD_T, w1c], axis=1)), F32R)      # [128,256]
    consts["w2y"] = (_f32(BD_T), F32R)
    dccor = np.zeros((128, 128))
    dccor[:, 0::8] = -8.0 * float(LEVEL) / 128.0
    consts["dccor"] = (_f32(dccor), F32R)
    pat8 = np.zeros((128, 512))
    pat8[:, 0::8] = 1.0
    consts["pat8"] = (_f32(pat8), F32R)
    consts["w2c"] = (_f32(PF), F32R)                          # [128,64]

    q = np.round(quantize[0].astype(np.float64) * 255.0) / 255.0
    rq = 1.0 / q
    consts["rqt2"] = (_f32(np.tile(rq.T, (16, 128))), F32)    # [128,1024]
    consts["qt2"] = (_f32(np.tile(q.T, (16, 128))), F32)

    consts["bd16"] = (_bf16(BD), BF16)
    consts["pu16"] = (_bf16(PU), BF16)                        # [64,128]
    consts["w4y16"] = (_bf16(BD), BF16)
    for name, cb, cr in (("R", CB_C[0], CR_C[0]), ("G", CB_C[1], CR_C[1]),
                         ("B", CB_C[2], CR_C[2])):
        m = np.zeros((128, 128)); m[0:64] = cb * PU; m[64:128] = cr * PU
        consts[f"w4c{name}"] = (_bf16(m), BF16)
    biasL = np.zeros((128, 1)); biasL[0::8, 0] = float(LEVEL) * np.sqrt(8.0)
    consts["biasL"] = (_f32(biasL), F32)
    consts["biasC"] = (_f32(np.full((128, 1), C_ROUND)), F32)
    consts["biasCn"] = (_f32(np.full((128, 1), -C_ROUND)), F32)
    return consts


_CONST_META = None


def _build_nc():
    nc = bacc.Bacc("TRN2", target_bir_lowering=False, debug=False,
                   enable_asserts=False, num_devices=N_CORES)
    x_d = nc.dram_tensor("x", [IMG_PER_CORE, 3, H, W], F32R,
                         kind="ExternalInput").ap()
    out_d = nc.dram_tensor("out", [IMG_PER_CORE, 3, H, W], F32,
                           kind="ExternalOutput").ap()
    cd = {}
    for name, (shape, dt) in _CONST_META.items():
        cd[name] = nc.dram_tensor(name, list(shape), dt,
                                  kind="ExternalInput").ap()

    ACT = mybir.ActivationFunctionType
    OP = mybir.AluOpType

    with tile.TileContext(nc) as tc:
        with tc.tile_pool(name="consts", bufs=1) as cp, \
             tc.tile_pool(name="xin", bufs=6) as xp, \
             tc.tile_pool(name="t1", bufs=6) as t1p, \
             tc.tile_pool(name="ey", bufs=4) as eyp, \
             tc.tile_pool(name="dec", bufs=4) as decp, \
             tc.tile_pool(name="t2", bufs=4) as t2p, \
             tc.tile_pool(name="og", bufs=6) as ogp, \
             tc.tile_pool(name="pa", bufs=2, space="PSUM") as pap, \
             tc.tile_pool(name="pb", bufs=2, space="PSUM") as pbp:

            cs = {}
            for name, (shape, dt) in _CONST_META.items():
                cs[name] = cp.tile(list(shape), dt, tag=f"c_{name}",
                                   name=f"c_{name}")
                nc.sync.dma_start(cs[name][:], cd[name])

            X = {}        # (img, c) -> [128, 2048] tile (cols 512*t + w)
            t1y = {}      # (img, s) -> [128,512]  [w, hfY]
            t1c = {}
            decy = {}     # (img, sp) -> [128,1024] bf16
            decc = {}     # img -> [128,1024] bf16
            t2y = {}      # (img, tp) -> [128,1024] bf16
            t2c = {}

            PLAIN_DMA = True

            def load(img):
                for c in range(3):
                    xt = xp.tile([128, 2048], F32R, tag="x",
                                 name=f"x_{img}_{c}")
                    if PLAIN_DMA:
                        for t in range(4):
                            nc.sync.dma_start(
                                xt[:, 512 * t:512 * (t + 1)],
                                x_d[img, c, 128 * t:128 * (t + 1), :])
                    else:
                        src = x_d[img, c].rearrange("(t p) w -> p t w", p=128)
                        dst = xt[:].rearrange("p (t w) -> p t w", w=512)
                        nc.sync.dma_start(dst, src)
                    X[img, c] = xt

            def s1(img):
                for s in range(4):
                    pa = pap.tile([128, 1024], F32, tag="pa", name="pa_t")
                    for t in range(4):
                        o = pa[:, 256 * t:256 * (t + 1)]
                        for c in range(3):
                            lhsT = X[img, c][:, 512 * t + 128 * s:
                                             512 * t + 128 * (s + 1)]
                            nc.tensor.matmul(o, lhsT, cs[f"r{c}"][:],
                                             start=(c == 0), stop=(c == 2))
                    ty = t1p.tile([128, 512], F32R, tag="t1y",
                                  name=f"t1y_{img}_{s}")
                    tcc = t1p.tile([128, 512], F32R, tag="t1c",
                                   name=f"t1c_{img}_{s}")
                    pav = pa[:].rearrange("p (t g) -> p t g", g=256)
                    nc.scalar.activation(
                        ty[:].rearrange("p (t h) -> p t h", h=128),
                        pav[:, :, 0:128], ACT.Copy)
                    nc.scalar.activation(
                        tcc[:].rearrange("p (t h) -> p t h", h=128),
                        pav[:, :, 128:256], ACT.Copy)
                    t1y[img, s] = ty
                    t1c[img, s] = tcc

            def p2q(img):
                # luma: two s-pairs, each a [128,1024] psum (2 banks)
                for sp in range(2):
                    pb = pbp.tile([128, 1024], F32, tag="pb", name="pb_t")
                    for s in (2 * sp, 2 * sp + 1):
                        o = pb[:, 512 * (s % 2):512 * (s % 2) + 512]
                        nc.tensor.matmul(o, cs["w2y"][:], t1y[img, s][:],
                                         start=True, stop=False)
                        nc.tensor.matmul(o, cs["dccor"][:], cs["pat8"][:],
                                         start=False, stop=True)
                    ey = eyp.tile([128, 1024], F32R, tag="ey",
                                  name=f"ey_{img}_{sp}")
                    nc.vector.tensor_tensor(ey[:], pb[:], cs["rqt2"][:],
                                            OP.mult)
                    nc.vector.tensor_scalar(ey[:], ey[:], C_ROUND,
                                            C_ROUND, OP.add, OP.subtract)
                    dy = decp.tile([128, 1024], BF16, tag="decy",
                                   name=f"decy_{img}_{sp}")
                    nc.vector.tensor_tensor(dy[:], ey[:], cs["qt2"][:],
                                            OP.mult)
                    decy[img, sp] = dy
                # chroma: per s-pair a [64,1024] psum (partitions 0:64)
                for sp in range(2):
                    pbf = pbp.tile([128, 1024], F32, tag="pb", name="pb_t")
                    pb = pbf[0:64, :]
                    for k, s in enumerate((2 * sp, 2 * sp + 1)):
                        o = pb[:, 512 * k:512 * k + 512]
                        nc.tensor.matmul(o, cs["w2c"][:], t1c[img, s][:],
                                         start=True, stop=True)
                    ec = eyp.tile([64, 1024], F32R, tag="ec",
                                  name=f"ec_{img}_{sp}")
                    nc.vector.tensor_tensor(ec[:], pb[:],
                                            cs["rqt2"][0:64, :], OP.mult)
                    nc.vector.tensor_scalar(ec[:], ec[:], C_ROUND,
                                            C_ROUND, OP.add, OP.subtract)
                    dc = decp.tile([64, 1024], BF16, tag="decc",
                                   name=f"decc_{img}_{sp}")
                    nc.vector.tensor_tensor(dc[:], ec[:],
                                            cs["qt2"][0:64, :], OP.mult)
                    decc[img, sp] = dc

            def t2p3(img):
                for tp in range(2):
                    pa = pap.tile([128, 1024], F32, tag="pa", name="pa_t")
                    for t in (2 * tp, 2 * tp + 1):
                        win = 512 * (t % 2)
                        for s in range(4):
                            lhsT = decy[img, s // 2][
                                :, 512 * (s % 2) + 128 * t:
                                512 * (s % 2) + 128 * (t + 1)]
                            nc.tensor.matmul(
                                pa[:, win + 128 * s:win + 128 * (s + 1)],
                                lhsT, cs["bd16"][:], start=True, stop=True)
                    sy = t2p.tile([128, 1024], BF16, tag="t2y",
                                  name=f"t2y_{img}_{tp}")
                    nc.scalar.activation(sy[:], pa[:], ACT.Identity,
                                         bias=cs["biasL"][:])
                    t2y[img, tp] = sy
                    pc = pap.tile([128, 1024], F32, tag="pa", name="pa_t")
                    for t in (2 * tp, 2 * tp + 1):
                        win = 512 * (t % 2)
                        for s in range(4):
                            lhsT = decc[img, s // 2][
                                :, 512 * (s % 2) + 128 * t:
                                512 * (s % 2) + 128 * (t + 1)]
                            nc.tensor.matmul(
                                pc[:, win + 128 * s:win + 128 * (s + 1)],
                                lhsT, cs["pu16"][:],
                                start=True, stop=True)
                    sc = t2p.tile([128, 1024], BF16, tag="t2c",
                                  name=f"t2c_{img}_{tp}")
                    nc.scalar.activation(sc[:], pc[:], ACT.Copy)
                    t2c[img, tp] = sc

            def p4(img):
                for t in range(4):
                    tp, win = t // 2, 512 * (t % 2)
                    ry = t2y[img, tp][:, win:win + 512]
                    rc = t2c[img, tp][:, win:win + 512]
                    og = ogp.tile([128, 1536], F32, tag="og",
                                  name=f"og_{img}_{t}")
                    pbA = pbp.tile([128, 1024], F32, tag="pb", name="pb_t")
                    for ci, cname in enumerate(("R", "G")):
                        o = pbA[:, 512 * ci:512 * (ci + 1)]
                        nc.tensor.matmul(o, cs["w4y16"][:], ry,
                                         start=True, stop=False)
                        nc.tensor.matmul(o, cs[f"w4c{cname}"][:], rc,
                                         start=False, stop=True)
                    pbB = pbp.tile([128, 1024], F32, tag="pb", name="pb_t")
                    o = pbB[:, 0:512]
                    nc.tensor.matmul(o, cs["w4y16"][:], ry,
                                     start=True, stop=False)
                    nc.tensor.matmul(o, cs["w4cB"][:], rc,
                                     start=False, stop=True)
                    nc.vector.tensor_scalar(og[:, 0:1024], pbA[:],
                                            0.0, 1.0, OP.max, OP.min)
                    nc.vector.tensor_scalar(og[:, 1024:1536], pbB[:, 0:512],
                                            0.0, 1.0, OP.max, OP.min)
                    if PLAIN_DMA:
                        for ci in range(3):
                            nc.sync.dma_start(
                                out_d[img, ci, 128 * t:128 * (t + 1), :],
                                og[:, 512 * ci:512 * (ci + 1)])
                    else:
                        dst = out_d[img, :, 128 * t:128 * (t + 1), :]\
                            .rearrange("c p w -> p c w")
                        src = og[:].rearrange("p (c w) -> p c w", w=512)
                        nc.sync.dma_start(dst, src)

            import os as _os
            _stage = int(_os.environ.get("BASS_DEBUG_STAGE", "0"))
            if _stage == 0:
                # software-pipelined emission
                load(0); s1(0); p2q(0)
                load(1); s1(1); t2p3(0); p2q(1); p4(0)
                load(2); s1(2); t2p3(1); p2q(2); p4(1)
                load(3); s1(3); t2p3(2); p2q(3); p4(2)
                t2p3(3); p4(3)
            elif _stage == 1:
                # serial emission
                for i in range(4):
                    load(i); s1(i); p2q(i); t2p3(i); p4(i)
            else:
                # truncated pipeline; dump intermediates raw for crash test
                for i in range(4):
                    load(i); s1(i)
                    if _stage >= 3:
                        p2q(i)
                    if _stage >= 4:
                        t2p3(i)
                    if _stage >= 5:
                        p4(i)
                    if _stage == 2:
                        for s in range(4):
                            nc.sync.dma_start(
                                out_d[i, 0, 128 * s:128 * (s + 1), :]
                                .bitcast(F32R),
                                t1y[i, s][:])
                    elif _stage == 3:
                        for sp in range(2):
                            nc.sync.dma_start(
                                out_d[i, 1, 128 * sp:128 * (sp + 1), :]
                                .bitcast(BF16),
                                decy[i, sp][:])
                        for sp in range(2):
                            nc.sync.dma_start(
                                out_d[i, 1, 256 + 64 * sp:256 + 64 * (sp + 1),
                                      :].bitcast(BF16),
                                decc[i, sp][:])
                    elif _stage == 4:
                        for tp in range(2):
                            nc.sync.dma_start(
                                out_d[i, 2, 128 * tp:128 * (tp + 1), :]
                                .bitcast(BF16),
                                t2y[i, tp][:])
                        for tp in range(2):
                            nc.sync.dma_start(
                                out_d[i, 2, 256 + 128 * tp:
                                      256 + 128 * (tp + 1), :].bitcast(BF16),
                                t2c[i, tp][:])

    nc.compile()
    return nc


_NC_CACHE = None


def kernel(input, quantize):
    global _NC_CACHE, _CONST_META
    input = np.asarray(input, dtype=np.float32)
    quantize = np.asarray(quantize, dtype=np.float32)
    consts = _build_consts(quantize)
    if _CONST_META is None:
        _CONST_META = {k: (v[0].shape, v[1]) for k, v in consts.items()}
    if _NC_CACHE is None:
        _NC_CACHE = _build_nc()
    nc = _NC_CACHE

    const_arrs = {k: v[0] for k, v in consts.items()}
    in_maps = []
    for core in range(N_CORES):
        shard = np.ascontiguousarray(
            input[core * IMG_PER_CORE:(core + 1) * IMG_PER_CORE])
        m = {"x": shard}
        m.update(const_arrs)
        in_maps.append(m)
    res = bass_utils.run_bass_kernel_spmd(nc, in_maps,
                                          core_ids=list(range(N_CORES)))
    global LAST_RESULT
    LAST_RESULT = res
    out = np.concatenate([res.results[i]["out"] for i in range(N_CORES)],
                         axis=0)
    return out.astype(np.float32)


LAST_RESULT = None
